# revision 4
# baseline (speedup 1.0000x reference)
"""PhysicsConsistency Trainium2 kernel.

Full input x[256, 8192, 16] f32 -> output [256, 8191, 20] f32.
Data parallel over batch: 8 cores x 32 rows each. Inside each core,
128 partitions = 4 seq-blocks x 32 rows; each partition owns a 2048-long
seq span of one row and processes it in NT tiles of L output positions,
loading a 21-element halo (20 back for the sliding window, 1 forward).

Per-tile pipeline (slot s of the x tile = seq t = T - 20 + s, output
j in [0, L) = seq t = T + j):
  ACT : y[:, :, 0:9]  = x[t] / DT       (pos, att, rate)
        y[:, :, 9:12] = x[t] / DT^2     (vel)
  DVE : tpa  = y+1 - y  (pos/att derivs);  tacc = y'+1 - y' (accel/DT)
        dpos = tpa[0:3] - vel+1 ; datt = tpa[3:6] - rate+1
        jerk = tacc+1 - tacc
        cs   = running-sum scan of raw vel   (3 channels)
        u    = DT*cs - pos+1 ;  kd = u+20 - u   (telescoped window sum)
        n2   = sq0+sq1 ; n2 += sq2
  GPS : angacc = y+1 - y (rate channels);  sq = v*v over all 5 groups
  ACT : out[norm ch] = sqrt(n2)  (single strided op, 5 channels)
"""

import numpy as np

DT = 0.005
W = 20
B_FULL = 256
SEQ = 8192
CIN = 16
COUT = 20
N_CORES = 8
ROWS = B_FULL // N_CORES  # 32 rows per core
NBLK = 4                  # seq blocks per row -> 128 partitions
BLK = SEQ // NBLK         # 2048
P = 128
L = 256                   # output positions per tile iteration
NT = BLK // L             # tile iterations
H = W + 1                 # halo elements per tile (20 back + 1 forward)

_nc_cache = None


def _c2(ap):
    """Squeeze a [P, N, 1]-ish AP view to 2D [P, N] for scan ops."""
    while ap.ndim > 2:
        ap = ap.squeeze(ap.ndim - 1)
    return ap


def build_nc():
    from contextlib import ExitStack
    from concourse import bass, bacc, mybir
    from concourse import tile

    f32 = mybir.dt.float32
    Alu = mybir.AluOpType

    nc = bacc.Bacc("TRN2", target_bir_lowering=False, debug=False)
    x = nc.dram_tensor("x", [ROWS, SEQ, CIN], f32, kind="ExternalInput")
    out = nc.dram_tensor("out", [ROWS, SEQ - 1, COUT], f32, kind="ExternalOutput")

    INV_DT = 1.0 / DT
    INV_DT2 = 1.0 / (DT * DT)

    with tile.TileContext(nc) as tc:
        with ExitStack() as ctx:
            xt_pool = ctx.enter_context(tc.tile_pool(name="xt", bufs=3))
            out_pool = ctx.enter_context(tc.tile_pool(name="outp", bufs=2))
            y_pool = ctx.enter_context(tc.tile_pool(name="y", bufs=2))
            tmp_pool = ctx.enter_context(tc.tile_pool(name="tmp", bufs=2))
            sq_pool = ctx.enter_context(tc.tile_pool(name="sq", bufs=1))
            const_pool = ctx.enter_context(tc.tile_pool(name="const", bufs=1))

            zt = const_pool.tile([P, L + H], f32)  # zeros: scan's data1
            nc.gpsimd.memset(zt[:], 0.0)

            for j in range(NT):
                first = j == 0
                last = j == NT - 1

                xt = xt_pool.tile([P, L + H, CIN], f32)
                # ---- load x tile (halo: 20 back, 1 forward) ----
                if first:
                    # block 0 (partitions 0:32): clamp at seq 0, zero halo
                    nc.sync.dma_start(
                        xt[0:32, W:, :], x.ap()[:, 0 : L + 1, :]
                    )
                    nc.gpsimd.memset(xt[0:32, 0:W, :], 0.0)
                    # blocks 1:4 read across their block boundary
                    src = bass.AP(
                        tensor=x,
                        offset=(BLK - W) * CIN,
                        ap=[
                            [BLK * CIN, NBLK - 1],
                            [SEQ * CIN, ROWS],
                            [CIN, L + H],
                            [1, CIN],
                        ],
                    )
                    nc.sync.dma_start(xt[32:P, :, :], src)
                elif last:
                    # blocks 0:3 full; block 3 clamps at seq end (misses +1)
                    src = bass.AP(
                        tensor=x,
                        offset=(j * L - W) * CIN,
                        ap=[
                            [BLK * CIN, NBLK - 1],
                            [SEQ * CIN, ROWS],
                            [CIN, L + H],
                            [1, CIN],
                        ],
                    )
                    nc.sync.dma_start(xt[0:96, :, :], src)
                    src2 = bass.AP(
                        tensor=x,
                        offset=((NBLK - 1) * BLK + j * L - W) * CIN,
                        ap=[
                            [SEQ * CIN, ROWS],
                            [CIN, L + H - 1],
                            [1, CIN],
                        ],
                    )
                    nc.sync.dma_start(xt[96:P, 0 : L + H - 1, :], src2)
                    nc.gpsimd.memset(xt[96:P, L + H - 1 : L + H, :], 0.0)
                else:
                    src = bass.AP(
                        tensor=x,
                        offset=(j * L - W) * CIN,
                        ap=[
                            [BLK * CIN, NBLK],
                            [SEQ * CIN, ROWS],
                            [CIN, L + H],
                            [1, CIN],
                        ],
                    )
                    nc.sync.dma_start(xt[:, :, :], src)

                ot = out_pool.tile([P, L, COUT], f32)
                # 5-group strided views over the out tile: [P, L, 5, 4]
                og = ot[:].rearrange("p l (g f) -> p l g f", g=5)
                ovec = og[:, :, :, 0:3]   # the 5 3-vectors
                onrm = og[:, :, :, 3:4].squeeze(3)  # [P, L, 5] norm chans

                # ---- scaled copies (ACT) ----
                # y slot k = x slot k+19  (covers t-1 .. t+L)
                y = y_pool.tile([P, L + 2, 12], f32)
                nc.scalar.mul(y[:, :, 0:9], xt[:, 19 : L + 21, 0:9], INV_DT)
                nc.scalar.mul(y[:, :, 9:12], xt[:, 19 : L + 21, 9:12], INV_DT2)

                # ---- first derivatives (DVE) ----
                tpa = tmp_pool.tile([P, L, 6], f32)  # pos/att derivs
                nc.vector.tensor_sub(
                    tpa[:], y[:, 2 : L + 2, 0:6], y[:, 1 : L + 1, 0:6]
                )
                # angular accel -> out ch 12:15 directly (GPSIMD)
                nc.gpsimd.tensor_sub(
                    ot[:, :, 12:15], y[:, 2 : L + 2, 6:9], y[:, 1 : L + 1, 6:9]
                )
                tacc = tmp_pool.tile([P, L + 1, 3], f32)  # accel / DT
                nc.vector.tensor_sub(
                    tacc[:], y[:, 1 : L + 2, 9:12], y[:, 0 : L + 1, 9:12]
                )

                # dpos = pos_deriv - vel[t+1]; datt = att_deriv - rate[t+1]
                nc.vector.tensor_sub(
                    ot[:, :, 0:3], tpa[:, :, 0:3], xt[:, W + 1 :, 9:12]
                )
                nc.vector.tensor_sub(
                    ot[:, :, 4:7], tpa[:, :, 3:6], xt[:, W + 1 :, 6:9]
                )
                # jerk = (accel[t] - accel[t-1]) / DT
                nc.vector.tensor_sub(
                    ot[:, :, 8:11], tacc[:, 1 : L + 1, :], tacc[:, 0:L, :]
                )

                # ---- sliding-window sum via local cumsum (DVE scan) ----
                cs = tmp_pool.tile([P, L + H, 3], f32)
                for c in range(3):
                    nc.vector.tensor_tensor_scan(
                        _c2(cs[:, :, c : c + 1].squeeze(2)),
                        _c2(xt[:, :, 9 + c : 10 + c].squeeze(2)),
                        zt[:],
                        0.0,
                        Alu.add,
                        Alu.add,
                    )
                # u[s] = DT*cs[s] - pos[s+1];  kd[j] = u[j+20] - u[j]
                u = tmp_pool.tile([P, L + W, 3], f32)
                nc.vector.scalar_tensor_tensor(
                    u[:],
                    cs[:, 0 : L + W, :],
                    DT,
                    xt[:, 1 : L + W + 1, 0:3],
                    Alu.mult,
                    Alu.subtract,
                )
                nc.vector.tensor_sub(
                    ot[:, :, 16:19], u[:, W : L + W, :], u[:, 0:L, :]
                )

                # ---- norms ----
                sq = sq_pool.tile([P, L, 5, 3], f32)
                nc.gpsimd.tensor_mul(sq[:], ovec, ovec)
                n2a = tmp_pool.tile([P, L, 5], f32)
                n2 = tmp_pool.tile([P, L, 5], f32)
                nc.vector.tensor_add(
                    n2a[:], sq[:, :, :, 0:1].squeeze(3), sq[:, :, :, 1:2].squeeze(3)
                )
                nc.vector.tensor_add(
                    n2[:], n2a[:], sq[:, :, :, 2:3].squeeze(3)
                )
                nc.scalar.sqrt(onrm, n2[:])

                # ---- zero-fix the windows the reference zero-pads ----
                if first:
                    # jerk_padded[0] = 0 (+ its norm), block-0 partitions
                    nc.gpsimd.memset(ot[0:32, 0:1, 8:12], 0.0)
                    # kd[0:W] = 0 (+ its norm)
                    nc.gpsimd.memset(ot[0:32, 0:W, 16:20], 0.0)

                # ---- store ----
                if not last:
                    dst = bass.AP(
                        tensor=out,
                        offset=j * L * COUT,
                        ap=[
                            [BLK * COUT, NBLK],
                            [(SEQ - 1) * COUT, ROWS],
                            [COUT, L],
                            [1, COUT],
                        ],
                    )
                    nc.scalar.dma_start(dst, ot[:])
                else:
                    dst = bass.AP(
                        tensor=out,
                        offset=j * L * COUT,
                        ap=[
                            [BLK * COUT, NBLK - 1],
                            [(SEQ - 1) * COUT, ROWS],
                            [COUT, L],
                            [1, COUT],
                        ],
                    )
                    nc.scalar.dma_start(dst, ot[0:96, :, :])
                    # block 3: final position t = SEQ-1 does not exist
                    dst2 = bass.AP(
                        tensor=out,
                        offset=((NBLK - 1) * BLK + j * L) * COUT,
                        ap=[
                            [(SEQ - 1) * COUT, ROWS],
                            [COUT, L - 1],
                            [1, COUT],
                        ],
                    )
                    nc.scalar.dma_start(dst2, ot[96:P, 0 : L - 1, :])

    nc.compile()
    return nc


def _get_nc():
    global _nc_cache
    if _nc_cache is None:
        _nc_cache = build_nc()
    return _nc_cache


def kernel(**inputs) -> np.ndarray:
    from concourse.bass_utils import run_bass_kernel_spmd

    x = np.ascontiguousarray(np.asarray(inputs["x"], dtype=np.float32))
    assert x.shape == (B_FULL, SEQ, CIN), x.shape

    nc = _get_nc()
    in_maps = [
        {"x": np.ascontiguousarray(x[i * ROWS : (i + 1) * ROWS])}
        for i in range(N_CORES)
    ]
    res = run_bass_kernel_spmd(nc, in_maps, list(range(N_CORES)))
    return np.concatenate(
        [res.results[i]["out"] for i in range(N_CORES)], axis=0
    )
